# revision 8
# baseline (speedup 1.0000x reference)
import os
import sys
import numpy as np

for _p in ("/opt/trn_rl_repo",):
    if _p not in sys.path:
        sys.path.insert(0, _p)

P, A, K = 200000, 100000, 50000
C, H, D, L = 128, 4, 32, 2
W = 8
PP, AA, KK = P // W, A // W, K // W          # 25000, 12500, 6250
PPp, AAp, KKp = 25088, 12544, 6272          # padded to 128
B0, B1, B2 = 0, PPp, PPp + AAp              # local (padded) type bases
NLP = PPp + AAp + KKp                        # 43904 local rows per core
TYPE_P = (PPp, AAp, KKp)
REL_META = ((0, 1, 0), (1, 0, 1), (2, 0, 0), (3, 0, 2))
SQRT_D = float(np.sqrt(D))
STRIDE = 64
# groups: per dst type; (dst_type, [relations])
GROUPS = ((0, (0, 2)), (1, (1,)), (2, (3,)))
# geometric window tiles per group (stride 64, window 128)
TGEO = tuple((TYPE_P[t] - 128) // 64 + 1 for t, _ in GROUPS)     # 391, 195, 97
PASSES = (1, 1, 2)
OVT = 8                                       # overflow tiles per (layer, group)
TUO = tuple(TYPE_P[t] // 128 for t, _ in GROUPS)                 # 196, 98, 49

_PLAN_CACHE = {}


def _dev_ids(t, idx):
    """global per-type index -> (rank, local padded row)"""
    if t == 0:
        return idx // PP, B0 + idx % PP
    if t == 1:
        return idx // AA, B1 + idx % AA
    return idx // KK, B2 + idx % KK


def _blockdiag(Wr):
    out = np.zeros((C, C), np.float32)
    for h in range(H):
        out[h * D:(h + 1) * D, h * D:(h + 1) * D] = Wr[h]
    return out


def _pack_group(dst_loc, order_payload, base_t, T, npass):
    """Greedy window packing. dst_loc sorted ascending (local rows incl type base).
    Returns (passes list of [T,128] slot->edge-idx (-1 pad)), overflow edge list."""
    reld = dst_loc - base_t
    E = len(dst_loc)
    slots = [np.full((T, 128), -1, np.int64) for _ in range(npass)]
    fill = [np.zeros(T, np.int64) for _ in range(npass)]
    overflow = []
    for p in range(npass):
        todo = overflow if p else range(E)
        if p:
            todo, overflow = overflow, []
        k = 0
        for e in todo:
            Dv = reld[e]
            kmin = max(0, (Dv - 127 + 63) // 64)
            kmax = min(T - 1, Dv // 64)
            kk = max(k, kmin)
            while kk <= kmax and fill[p][kk] >= 128:
                kk += 1
            if kk > kmax:
                overflow.append(e)
                continue
            k = kk
            slots[p][kk, fill[p][kk]] = e
            fill[p][kk] += 1
    return slots, overflow


def _host_prep(inputs):
    inp = {k: np.asarray(v) for k, v in inputs.items()}
    x_full = [inp["x_paper"].astype(np.float32), inp["x_author"].astype(np.float32),
              inp["x_keyword"].astype(np.float32)]
    edges_raw = ((inp["src_writes"], inp["dst_writes"]), (inp["src_wb"], inp["dst_wb"]),
                 (inp["src_cites"], inp["dst_cites"]), (inp["src_has"], inp["dst_has"]))
    Wkqv = inp["Wkqv"].astype(np.float32)
    bkqv = inp["bkqv"].astype(np.float32)
    Wk_rel = inp["Wk_rel"].astype(np.float32)
    Wv_rel = inp["Wv_rel"].astype(np.float32)
    p_rel = inp["p_rel"].astype(np.float32)

    # --- per-relation device edge arrays ---
    # src dev id (rank, locp) -> global dev row = rank*NLP + locp
    rel_edges = []
    for (r, st, dt) in REL_META:
        s, d = edges_raw[r]
        s = np.asarray(s, np.int64)
        d = np.asarray(d, np.int64)
        sr, sl = _dev_ids(st, s)
        dr, dl = _dev_ids(dt, d)
        src_dev = sr * NLP + sl
        rel_edges.append((src_dev, dr, dl))

    # --- weights folded ---
    # per (l, r): Mk = Wk[l,st] @ BD(Wk_rel) * headscale ; bkr likewise
    wm, wb = {}, {}
    for t in range(3):
        wm[("win", t)] = inp["W_in"].astype(np.float32)[t]
        wb[("bin", t)] = inp["b_in"].astype(np.float32)[t]
    for l in range(L):
        for t in range(3):
            wm[("wq", l, t)] = Wkqv[l, t][:, C:2 * C].copy()
            wb[("bq", l, t)] = bkqv[l, t][C:2 * C].copy()
            wm[("wout", l, t)] = inp["Wout"].astype(np.float32)[l, t]
            wb[("bout", l, t)] = inp["bout"].astype(np.float32)[l, t]
        for (r, st, dt) in REL_META:
            scale = np.repeat(p_rel[l, r] / SQRT_D, D)           # [C] per-head col scale
            Mk = (Wkqv[l, st][:, :C] @ _blockdiag(Wk_rel[l, r])) * scale[None, :]
            bk = (bkqv[l, st][:C] @ _blockdiag(Wk_rel[l, r])) * scale
            Mv = Wkqv[l, st][:, 2 * C:] @ _blockdiag(Wv_rel[l, r])
            bv = bkqv[l, st][2 * C:] @ _blockdiag(Wv_rel[l, r])
            wm[("mk", l, r)] = Mk
            wb[("bk", l, r)] = bk
            wm[("mv", l, r)] = Mv
            wb[("bv", l, r)] = bv
    sg = 1.0 / (1.0 + np.exp(-inp["skip"].astype(np.float32)))   # [L,3]

    # --- per-core packing ---
    cores = []
    for c in range(W):
        core = {"edges": {}}
        for gi, (dt_t, rels) in enumerate(GROUPS):
            base_t = (B0, B1, B2)[dt_t]
            # gather this core's edges of the group's relations
            srcs, dls, rids = [], [], []
            for r in rels:
                src_dev, dr, dl = rel_edges[r]
                m = dr == c
                srcs.append(src_dev[m])
                dls.append(dl[m])
                rids.append(np.full(m.sum(), r, np.int64))
            srcs = np.concatenate(srcs)
            dls = np.concatenate(dls)
            rids = np.concatenate(rids)
            o = np.argsort(dls, kind="stable")
            srcs, dls, rids = srcs[o], dls[o], rids[o]
            T = TGEO[gi]
            slots, overflow = _pack_group(dls, None, base_t, T, PASSES[gi])
            core["edges"][gi] = (srcs, dls, rids, slots, np.asarray(overflow, np.int64))
        cores.append(core)

    # --- layer-1 compact ext table ---
    ext_sets = []
    for c in range(W):
        need = []
        for gi in range(3):
            srcs = cores[c]["edges"][gi][0]
            need.append(srcs)
        need = np.unique(np.concatenate(need)) if need else np.zeros(0, np.int64)
        own = (need // NLP) == c
        ext_sets.append(need[~own])
    EXT_BY_T = []
    for tt, (lo, hi) in enumerate(((B0, B0 + PP), (B1, B1 + AA), (B2, B2 + KK))):
        mx = 0
        for c in range(W):
            loc = ext_sets[c] % NLP
            mx = max(mx, int(((loc >= lo) & (loc < hi)).sum()))
        EXT_BY_T.append(-(-mx // 128) * 128)
    EP, EA, EK = EXT_BY_T
    M0 = NLP + EP + EA + EK

    # per-core: ext id maps and x_needed
    x_cat = np.concatenate([x_full[0], x_full[1], x_full[2]], 0)  # [N, C] typed order

    def glob_row(dev):
        """device id -> row in x_cat (original global typed index)"""
        rank = dev // NLP
        loc = dev % NLP
        out = np.empty(len(dev), np.int64)
        mp = loc < B1
        ma = (loc >= B1) & (loc < B2)
        mk = loc >= B2
        out[mp] = rank[mp] * PP + (loc[mp] - B0)
        out[ma] = P + rank[ma] * AA + (loc[ma] - B1)
        out[mk] = P + A + rank[mk] * KK + (loc[mk] - B2)
        return out

    for c in range(W):
        ext = ext_sets[c]
        loc = ext % NLP
        exts = [ext[(loc >= B0) & (loc < B0 + PP)], ext[(loc >= B1) & (loc < B1 + AA)],
                ext[(loc >= B2) & (loc < B2 + KK)]]
        # compact id of ext row
        cid = {}
        base = NLP
        for tt in range(3):
            for i, dev in enumerate(exts[tt]):
                cid[int(dev)] = base + i
            base += EXT_BY_T[tt]
        cores[c]["extlists"] = exts
        cores[c]["extmap"] = cid
        xn = np.zeros((M0, C), np.float32)
        own_glob = np.empty(NLP, np.int64)
        own_glob[:] = -1
        own_glob[B0:B0 + PP] = c * PP + np.arange(PP)
        own_glob[B1:B1 + AA] = P + c * AA + np.arange(AA)
        own_glob[B2:B2 + KK] = P + A + c * KK + np.arange(KK)
        mreal = own_glob >= 0
        xn[:NLP][mreal] = x_cat[own_glob[mreal]]
        base = NLP
        for tt in range(3):
            if len(exts[tt]):
                xn[base:base + len(exts[tt])] = x_cat[glob_row(exts[tt])]
            base += EXT_BY_T[tt]
        cores[c]["x_needed"] = xn

    # --- per-(layer, group) uploaded index/aux arrays ---
    for c in range(W):
        core = cores[c]
        cid = core["extmap"]
        for l in range(L):
            for gi, (dt_t, rels) in enumerate(GROUPS):
                srcs, dls, rids, slots, ovf = core["edges"][gi]
                base_t = (B0, B1, B2)[dt_t]
                npass = PASSES[gi]
                T = TGEO[gi]
                sidx = np.zeros((npass, T, 128), np.int32)
                aux = np.zeros((npass, T, 128, 4), np.float32)
                for p in range(npass):
                    sl = slots[p]
                    m = sl >= 0
                    e = sl[m]
                    if l == 0:
                        sv = np.array([cid[int(s)] if (s // NLP) != c else int(s % NLP)
                                       for s in srcs[e]], np.int32)
                    else:
                        sv = srcs[e].astype(np.int32)
                    sidx[p][m] = sv
                    kidx = np.nonzero(m)[0] if False else None
                    # reldst per slot
                    tk = np.repeat(np.arange(T), 128).reshape(T, 128)
                    aux[p][..., 0][m] = (dls[e] - base_t) - 64 * tk[m]
                    if gi == 0:
                        aux[p][..., 1][m] = (rids[e] == 0).astype(np.float32)  # writes
                        aux[p][..., 2][m] = (rids[e] == 2).astype(np.float32)  # cites
                    else:
                        aux[p][..., 1][m] = 1.0
                    aux[p][..., 3][m] = 1.0
                core[f"src_{l}_{gi}"] = sidx.reshape(npass * T * 128, 1)
                core[f"aux_{l}_{gi}"] = aux.reshape(npass * T * 128, 4)
                # overflow tiles
                osrc = np.zeros((OVT * 128, 1), np.int32)
                oaux = np.zeros((OVT * 128, 4), np.float32)
                odst = np.full((OVT * 128, 1), 1 << 30, np.int32)
                ne = len(ovf)
                assert ne <= OVT * 128, f"overflow {ne} > {OVT*128}"
                if ne:
                    e = ovf
                    if l == 0:
                        sv = np.array([cid[int(s)] if (s // NLP) != c else int(s % NLP)
                                       for s in srcs[e]], np.int32)
                    else:
                        sv = srcs[e].astype(np.int32)
                    osrc[:ne, 0] = sv
                    # rank of dst within each tile of 128
                    for tb in range(0, ne, 128):
                        seg = e[tb:tb + 128]
                        dd = dls[seg]
                        uq, rk = np.unique(dd, return_inverse=True)
                        oaux[tb:tb + len(seg), 0] = rk
                        odst[tb:tb + len(seg), 0] = 0  # placeholder, fixed below
                        # dst row for rank i of this tile:
                        for i, dv in enumerate(uq):
                            pass
                        # store per-slot dst of its rank row later via scatter rows
                    # build odst per tile: row d -> U row
                    for tb in range(0, ne, 128):
                        seg = e[tb:tb + 128]
                        dd = dls[seg]
                        uq, rk = np.unique(dd, return_inverse=True)
                        od = np.full(128, 1 << 30, np.int32)
                        od[:len(uq)] = uq.astype(np.int32)
                        odst[tb:tb + 128, 0] = od
                        oaux[tb:tb + len(seg), 0] = rk
                    if gi == 0:
                        oaux[:ne, 1] = (rids[ovf] == 0).astype(np.float32)
                        oaux[:ne, 2] = (rids[ovf] == 2).astype(np.float32)
                    else:
                        oaux[:ne, 1] = 1.0
                    oaux[:ne, 3] = 1.0
                core[f"osrc_{l}_{gi}"] = osrc
                core[f"oaux_{l}_{gi}"] = oaux
                core[f"odst_{l}_{gi}"] = odst

    # --- stacked weight uploads ---
    mat_keys = [("win", t) for t in range(3)]
    for l in range(L):
        mat_keys += [("wq", l, t) for t in range(3)] + [("wout", l, t) for t in range(3)]
        for r in range(4):
            mat_keys += [("mk", l, r), ("mv", l, r)]
    bias_keys = [("bin", t) for t in range(3)]
    for l in range(L):
        bias_keys += [("bq", l, t) for t in range(3)] + [("bout", l, t) for t in range(3)]
        for r in range(4):
            bias_keys += [("bk", l, r), ("bv", l, r)]
    wmats = np.stack([wm[k] for k in mat_keys]).astype(np.float32)
    wbias = np.stack([wb[k] for k in bias_keys]).astype(np.float32)
    mat_idx = {k: i for i, k in enumerate(mat_keys)}
    bias_idx = {k: i for i, k in enumerate(bias_keys)}

    return dict(cores=cores, wmats=wmats, wbias=wbias, mat_idx=mat_idx,
                bias_idx=bias_idx, sg=sg, M0=M0, EXT=EXT_BY_T)


def _build(prep):
    import concourse.bacc as bacc
    import concourse.tile as tile
    import concourse.bass as bass
    from concourse import mybir
    from concourse.masks import make_identity
    from contextlib import ExitStack

    F32 = mybir.dt.float32
    I32 = mybir.dt.int32
    AOP = mybir.AluOpType
    ACT = mybir.ActivationFunctionType
    M0 = prep["M0"]
    NW = prep["wmats"].shape[0]
    NB = prep["wbias"].shape[0]
    mat_idx, bias_idx = prep["mat_idx"], prep["bias_idx"]
    sgv = prep["sg"]

    nc = bacc.Bacc("TRN2", target_bir_lowering=False, debug=False, num_devices=W)
    t_xn = nc.dram_tensor("x_needed", [M0, C], F32, kind="ExternalInput").ap()
    t_wm = nc.dram_tensor("wmats", [NW, C, C], F32, kind="ExternalInput").ap()
    t_wb = nc.dram_tensor("wbias", [NB, C], F32, kind="ExternalInput").ap()
    t_src, t_aux, t_osrc, t_oaux, t_odst = {}, {}, {}, {}, {}
    for l in range(L):
        for gi in range(3):
            n = PASSES[gi] * TGEO[gi] * 128
            t_src[(l, gi)] = nc.dram_tensor(f"src_{l}_{gi}", [n, 1], I32, kind="ExternalInput").ap()
            t_aux[(l, gi)] = nc.dram_tensor(f"aux_{l}_{gi}", [n, 4], F32, kind="ExternalInput").ap()
            t_osrc[(l, gi)] = nc.dram_tensor(f"osrc_{l}_{gi}", [OVT * 128, 1], I32, kind="ExternalInput").ap()
            t_oaux[(l, gi)] = nc.dram_tensor(f"oaux_{l}_{gi}", [OVT * 128, 4], F32, kind="ExternalInput").ap()
            t_odst[(l, gi)] = nc.dram_tensor(f"odst_{l}_{gi}", [OVT * 128, 1], I32, kind="ExternalInput").ap()

    t_x0 = nc.dram_tensor("X0c", [M0, C], F32).ap()
    t_x1l = nc.dram_tensor("X1loc", [NLP, C], F32).ap()
    t_x1f = nc.dram_tensor("X1full", [W * NLP, C], F32, addr_space="Shared").ap()
    SB_OFF = []
    _off = 0
    for gi in range(3):
        SB_OFF.append(_off)
        _off += PASSES[gi] * (TGEO[gi] + 2)
    SLABN = _off
    t_slab = nc.dram_tensor("slab", [SLABN * 128, 132], F32).ap()
    t_u = nc.dram_tensor("U", [NLP, 132], F32).ap()
    t_out = nc.dram_tensor("X2out", [NLP, C], F32, kind="ExternalOutput").ap()

    with tile.TileContext(nc) as tc:
        st = ExitStack()
        wpool = st.enter_context(tc.tile_pool(name="w", bufs=1))
        pool = st.enter_context(tc.tile_pool(name="p", bufs=6))
        big = st.enter_context(tc.tile_pool(name="b", bufs=6))
        pp_tp = st.enter_context(tc.tile_pool(name="tp", bufs=2, space="PSUM"))
        pp_mm = st.enter_context(tc.tile_pool(name="mm", bufs=2, space="PSUM"))
        pp_kr = st.enter_context(tc.tile_pool(name="kr", bufs=1, space="PSUM"))
        pp_al = st.enter_context(tc.tile_pool(name="al", bufs=1, space="PSUM"))
        pp_vr = st.enter_context(tc.tile_pool(name="vr", bufs=2, space="PSUM"))

        ident = wpool.tile([128, 128], F32)
        make_identity(nc, ident[:])
        iota = wpool.tile([128, 128], F32)
        nc.gpsimd.iota(iota[:], pattern=[[1, 128]], base=0, channel_multiplier=0,
                       allow_small_or_imprecise_dtypes=True)
        ones = wpool.tile([1, 128], F32)
        nc.vector.memset(ones[:], 1.0)
        hm = wpool.tile([128, 4], F32)
        nc.vector.memset(hm[:], 0.0)
        for h in range(H):
            nc.vector.memset(hm[32 * h:32 * (h + 1), h:h + 1], 1.0)
        zt = wpool.tile([128, 132], F32)
        nc.vector.memset(zt[:], 0.0)

        WM, WB = {}, {}
        for k, i in mat_idx.items():
            WM[k] = wpool.tile([128, 128], F32, tag=f"wm{i}", name=f"wm{i}")
            nc.sync.dma_start(WM[k][:], t_wm[i, :, :])
        for k, i in bias_idx.items():
            WB[k] = wpool.tile([1, 128], F32, tag=f"wb{i}", name=f"wb{i}")
            nc.sync.dma_start(WB[k][:], t_wb[i:i + 1, :])

        # zero slab guard tiles (per group/pass layout: base p*(T+2))
        for gi in range(3):
            Tg = TGEO[gi]
            for p in range(PASSES[gi]):
                b = SB_OFF[gi] + p * (Tg + 2)
                nc.sync.dma_start(t_slab[b * 128:(b + 1) * 128, :], zt[:])
                nc.sync.dma_start(t_slab[(b + Tg + 1) * 128:(b + Tg + 2) * 128, :], zt[:])

        # ---------- input phase ----------
        # own rows per type + ext rows per type, all 128-aligned
        blocks = [(0, 0, PPp // 128), (1, PPp // 128, AAp // 128), (2, B2 // 128, KKp // 128)]
        base = NLP // 128
        for tt, ext_n in enumerate(prep["EXT"]):
            if ext_n:
                blocks.append((tt, base, ext_n // 128))
            base += ext_n // 128
        for (tt, tb, ntl) in blocks:
            def body(i, tt=tt, tb=tb):
                xin = big.tile([128, 128], F32, tag="li_x")
                nc.sync.dma_start(xin[:], t_xn[bass.ds((i + tb) * 128, 128), :])
                xtp = pp_tp.tile([128, 128], F32, tag="t")
                nc.tensor.transpose(xtp[:], xin[:], ident[:])
                xts = big.tile([128, 128], F32, tag="li_xt")
                nc.vector.tensor_copy(out=xts[:], in_=xtp[:])
                mmp = pp_mm.tile([128, 128], F32, tag="m")
                nc.tensor.matmul(mmp[:], WM[("win", tt)][:], xts[:], start=True, stop=False)
                nc.tensor.matmul(mmp[:], WB[("bin", tt)][:1, :], ones[:1, :], start=False, stop=True)
                res = big.tile([128, 128], F32, tag="li_r")
                nc.scalar.activation(res[:], mmp[:], ACT.Relu)
                outp = pp_tp.tile([128, 128], F32, tag="t")
                nc.tensor.transpose(outp[:], res[:], ident[:])
                outs = big.tile([128, 128], F32, tag="li_o")
                nc.vector.tensor_copy(out=outs[:], in_=outp[:])
                nc.sync.dma_start(t_x0[bass.ds((i + tb) * 128, 128), :], outs[:])
            tc.For_i_unrolled(0, ntl, 1, body, max_unroll=4)

        # ---------- per-layer ----------
        for l in range(L):
            src_tab = t_x0 if l == 0 else t_x1f      # gather source (full ids)
            loc_tab = t_x0 if l == 0 else t_x1l      # dst windows / blend source

            for gi, (dt_t, rels) in enumerate(GROUPS):
                base_t = (B0, B1, B2)[dt_t]
                Tg = TGEO[gi]
                two = (gi == 0)
                r_a = rels[0]
                r_b = rels[1] if two else rels[0]

                def edge_body(i, p, l=l, gi=gi, dt_t=dt_t, base_t=base_t, Tg=Tg,
                              two=two, r_a=r_a, r_b=r_b, src_tab=src_tab, loc_tab=loc_tab):
                    off = p * Tg * 128
                    sidx = pool.tile([128, 1], I32, tag="e_si")
                    nc.sync.dma_start(sidx[:], t_src[(l, gi)][bass.ds(off + i * 128, 128), :])
                    aux = pool.tile([128, 4], F32, tag="e_aux")
                    nc.sync.dma_start(aux[:], t_aux[(l, gi)][bass.ds(off + i * 128, 128), :])
                    xwin = big.tile([128, 128], F32, tag="e_xw")
                    nc.sync.dma_start(xwin[:], loc_tab[bass.ds(base_t + i * 64, 128), :])
                    xs = big.tile([128, 128], F32, tag="e_xs")
                    nc.gpsimd.indirect_dma_start(
                        out=xs[:], out_offset=None, in_=src_tab[:, :],
                        in_offset=bass.IndirectOffsetOnAxis(ap=sidx[:, :1], axis=0))
                    # SELrelT [e, d] ; SELrel [d(win row), e]
                    selT = big.tile([128, 128], F32, tag="e_selT")
                    nc.vector.tensor_tensor(out=selT[:], in0=aux[:, 0:1].to_broadcast([128, 128]),
                                            in1=iota[:], op=AOP.is_equal)
                    selP = pp_tp.tile([128, 128], F32, tag="t")
                    nc.tensor.transpose(selP[:], selT[:], ident[:])
                    sel = big.tile([128, 128], F32, tag="e_sel")
                    nc.vector.tensor_copy(out=sel[:], in_=selP[:])
                    # XdTsel [c, e] = xwin.T selected
                    xdP = pp_mm.tile([128, 128], F32, tag="m")
                    nc.tensor.matmul(xdP[:], xwin[:], sel[:], start=True, stop=True)
                    xdT = big.tile([128, 128], F32, tag="e_xdT")
                    nc.vector.tensor_copy(out=xdT[:], in_=xdP[:])
                    # qbT [f, e]
                    qbP = pp_mm.tile([128, 128], F32, tag="m")
                    nc.tensor.matmul(qbP[:], WM[("wq", l, dt_t)][:], xdT[:], start=True, stop=False)
                    nc.tensor.matmul(qbP[:], WB[("bq", l, dt_t)][:1, :], ones[:1, :], start=False, stop=True)
                    qb_s = big.tile([128, 128], F32, tag="e_qbs")
                    nc.vector.tensor_copy(out=qb_s[:], in_=qbP[:])
                    # XsT
                    xsP = pp_tp.tile([128, 128], F32, tag="t")
                    nc.tensor.transpose(xsP[:], xs[:], ident[:])
                    xsT = big.tile([128, 128], F32, tag="e_xsT")
                    nc.vector.tensor_copy(out=xsT[:], in_=xsP[:])
                    # relation a
                    krP = pp_kr.tile([128, 128], F32, tag="k")
                    nc.tensor.matmul(krP[:], WM[("mk", l, r_a)][:], xsT[:], start=True, stop=False)
                    nc.tensor.matmul(krP[:], WB[("bk", l, r_a)][:1, :], ones[:1, :], start=False, stop=True)
                    qk = big.tile([128, 128], F32, tag="e_qk")
                    nc.vector.tensor_mul(out=qk[:], in0=qb_s[:], in1=krP[:])
                    alP = pp_al.tile([128, 4], F32, tag="a")
                    nc.tensor.matmul(alP[:], qk[:], hm[:], start=True, stop=True)
                    asel = pool.tile([128, 4], F32, tag="e_as")
                    nc.vector.tensor_scalar(out=asel[:], in0=alP[:], scalar1=aux[:, 1:2],
                                            scalar2=None, op0=AOP.mult)
                    vrPa = pp_vr.tile([128, 128], F32, tag="v")
                    nc.tensor.matmul(vrPa[:], xsT[:], WM[("mv", l, r_a)][:], start=True, stop=False)
                    nc.tensor.matmul(vrPa[:], ones[:1, :], WB[("bv", l, r_a)][:1, :], start=False, stop=True)
                    vsel = big.tile([128, 128], F32, tag="e_vs")
                    if two:
                        krP2 = pp_kr.tile([128, 128], F32, tag="k")
                        nc.tensor.matmul(krP2[:], WM[("mk", l, r_b)][:], xsT[:], start=True, stop=False)
                        nc.tensor.matmul(krP2[:], WB[("bk", l, r_b)][:1, :], ones[:1, :], start=False, stop=True)
                        qk2 = big.tile([128, 128], F32, tag="e_qk2")
                        nc.vector.tensor_mul(out=qk2[:], in0=qb_s[:], in1=krP2[:])
                        alP2 = pp_al.tile([128, 4], F32, tag="a")
                        nc.tensor.matmul(alP2[:], qk2[:], hm[:], start=True, stop=True)
                        asel2 = pool.tile([128, 4], F32, tag="e_as2")
                        nc.vector.tensor_scalar(out=asel2[:], in0=alP2[:], scalar1=aux[:, 2:3],
                                                scalar2=None, op0=AOP.mult)
                        nc.vector.tensor_add(out=asel[:], in0=asel[:], in1=asel2[:])
                        vrPb = pp_vr.tile([128, 128], F32, tag="v")
                        nc.tensor.matmul(vrPb[:], xsT[:], WM[("mv", l, r_b)][:], start=True, stop=False)
                        nc.tensor.matmul(vrPb[:], ones[:1, :], WB[("bv", l, r_b)][:1, :], start=False, stop=True)
                        nc.vector.tensor_scalar(out=vsel[:], in0=vrPa[:], scalar1=aux[:, 1:2],
                                                scalar2=None, op0=AOP.mult)
                        vsb = big.tile([128, 128], F32, tag="e_vsb")
                        nc.vector.tensor_scalar(out=vsb[:], in0=vrPb[:], scalar1=aux[:, 2:3],
                                                scalar2=None, op0=AOP.mult)
                        nc.vector.tensor_add(out=vsel[:], in0=vsel[:], in1=vsb[:])
                    else:
                        nc.vector.tensor_copy(out=vsel[:], in_=vrPa[:])
                    ea = pool.tile([128, 4], F32, tag="e_ea")
                    nc.scalar.activation(ea[:], asel[:], ACT.Exp)
                    nc.vector.tensor_scalar(out=ea[:], in0=ea[:], scalar1=aux[:, 3:4],
                                            scalar2=None, op0=AOP.mult)
                    wv = big.tile([128, 132], F32, tag="e_wv")
                    for h in range(H):
                        nc.vector.tensor_scalar(out=wv[:, 32 * h:32 * (h + 1)],
                                                in0=vsel[:, 32 * h:32 * (h + 1)],
                                                scalar1=ea[:, h:h + 1], scalar2=None, op0=AOP.mult)
                    nc.vector.tensor_copy(out=wv[:, 128:132], in_=ea[:])
                    aggP = pp_mm.tile([128, 132], F32, tag="m")
                    nc.tensor.matmul(aggP[:], selT[:], wv[:], start=True, stop=True)
                    agg = big.tile([128, 132], F32, tag="e_agg")
                    nc.vector.tensor_copy(out=agg[:], in_=aggP[:])
                    sb = SB_OFF[gi] + p * (Tg + 2) + 1
                    nc.sync.dma_start(t_slab[bass.ds((sb + i) * 128, 128), :], agg[:])

                for p in range(PASSES[gi]):
                    tc.For_i_unrolled(0, Tg, 1, lambda i, p=p: edge_body(i, p), max_unroll=4)

                # ---- reduce slab -> U region ----
                TU = TUO[gi]
                def red_body(j, gi=gi, base_t=base_t, Tg=Tg):
                    acc = big.tile([128, 132], F32, tag="r_acc")
                    first = True
                    for p in range(PASSES[gi]):
                        sb = SB_OFF[gi] + p * (Tg + 2)
                        a = big.tile([128, 132], F32, tag="r_a")
                        nc.sync.dma_start(a[:], t_slab[bass.ds((sb + 1 + j * 2) * 128, 128), :])
                        bt = big.tile([128, 132], F32, tag="r_b")
                        nc.sync.dma_start(bt[0:64, :], t_slab[bass.ds((sb + j * 2) * 128 + 64, 64), :])
                        nc.sync.dma_start(bt[64:128, :], t_slab[bass.ds((sb + 2 + j * 2) * 128, 64), :])
                        if first:
                            nc.vector.tensor_copy(out=acc[:], in_=a[:])
                            first = False
                        else:
                            nc.vector.tensor_add(out=acc[:], in0=acc[:], in1=a[:])
                        nc.vector.tensor_add(out=acc[:], in0=acc[:], in1=bt[:])
                    nc.sync.dma_start(t_u[bass.ds(base_t + j * 128, 128), :], acc[:])
                tc.For_i_unrolled(0, TU, 1, red_body, max_unroll=4)

                # ---- overflow tiles: indirect accumulate into U ----
                def ov_body(i, l=l, gi=gi, dt_t=dt_t, two=two, r_a=r_a, r_b=r_b, src_tab=src_tab):
                    sidx = pool.tile([128, 1], I32, tag="e_si")
                    nc.sync.dma_start(sidx[:], t_osrc[(l, gi)][bass.ts(i, 128), :])
                    aux = pool.tile([128, 4], F32, tag="e_aux")
                    nc.sync.dma_start(aux[:], t_oaux[(l, gi)][bass.ts(i, 128), :])
                    didx = pool.tile([128, 1], I32, tag="e_di")
                    nc.sync.dma_start(didx[:], t_odst[(l, gi)][bass.ts(i, 128), :])
                    xs = big.tile([128, 128], F32, tag="e_xs")
                    nc.gpsimd.indirect_dma_start(
                        out=xs[:], out_offset=None, in_=src_tab[:, :],
                        in_offset=bass.IndirectOffsetOnAxis(ap=sidx[:, :1], axis=0))
                    # dst rows gather for q
                    xd = big.tile([128, 128], F32, tag="e_xw")
                    nc.gpsimd.indirect_dma_start(
                        out=xd[:], out_offset=None, in_=t_x0[:, :] if l == 0 else t_x1l[:, :],
                        in_offset=bass.IndirectOffsetOnAxis(ap=didx[:, :1], axis=0),
                        bounds_check=NLP - 1, oob_is_err=False)
                    selT = big.tile([128, 128], F32, tag="e_selT")
                    nc.vector.tensor_tensor(out=selT[:], in0=aux[:, 0:1].to_broadcast([128, 128]),
                                            in1=iota[:], op=AOP.is_equal)
                    xdP = pp_tp.tile([128, 128], F32, tag="t")
                    nc.tensor.transpose(xdP[:], xd[:], ident[:])
                    xdT = big.tile([128, 128], F32, tag="e_xdT")
                    nc.vector.tensor_copy(out=xdT[:], in_=xdP[:])
                    # q rows [d, f] = xd @ Wq + bq; then qeT [f, e] = sum_d qb[d,f] selT[d->? sel]
                    qbP = pp_mm.tile([128, 128], F32, tag="m")
                    nc.tensor.matmul(qbP[:], xdT[:], WM[("wq", l, dt_t)][:], start=True, stop=False)
                    nc.tensor.matmul(qbP[:], ones[:1, :], WB[("bq", l, dt_t)][:1, :], start=False, stop=True)
                    qbp_s = big.tile([128, 128], F32, tag="e_qbs")
                    nc.vector.tensor_copy(out=qbp_s[:], in_=qbP[:])
                    selP = pp_tp.tile([128, 128], F32, tag="t")
                    nc.tensor.transpose(selP[:], selT[:], ident[:])
                    sel = big.tile([128, 128], F32, tag="e_sel")
                    nc.vector.tensor_copy(out=sel[:], in_=selP[:])
                    qeP = pp_mm.tile([128, 128], F32, tag="m")
                    nc.tensor.matmul(qeP[:], qbp_s[:], sel[:], start=True, stop=True)  # [f, e]
                    qe_s = big.tile([128, 128], F32, tag="e_qes")
                    nc.vector.tensor_copy(out=qe_s[:], in_=qeP[:])
                    xsP = pp_tp.tile([128, 128], F32, tag="t")
                    nc.tensor.transpose(xsP[:], xs[:], ident[:])
                    xsT = big.tile([128, 128], F32, tag="e_xsT")
                    nc.vector.tensor_copy(out=xsT[:], in_=xsP[:])
                    krP = pp_kr.tile([128, 128], F32, tag="k")
                    nc.tensor.matmul(krP[:], WM[("mk", l, r_a)][:], xsT[:], start=True, stop=False)
                    nc.tensor.matmul(krP[:], WB[("bk", l, r_a)][:1, :], ones[:1, :], start=False, stop=True)
                    qk = big.tile([128, 128], F32, tag="e_qk")
                    nc.vector.tensor_mul(out=qk[:], in0=qe_s[:], in1=krP[:])
                    alP = pp_al.tile([128, 4], F32, tag="a")
                    nc.tensor.matmul(alP[:], qk[:], hm[:], start=True, stop=True)
                    asel = pool.tile([128, 4], F32, tag="e_as")
                    nc.vector.tensor_scalar(out=asel[:], in0=alP[:], scalar1=aux[:, 1:2],
                                            scalar2=None, op0=AOP.mult)
                    vrPa = pp_vr.tile([128, 128], F32, tag="v")
                    nc.tensor.matmul(vrPa[:], xsT[:], WM[("mv", l, r_a)][:], start=True, stop=False)
                    nc.tensor.matmul(vrPa[:], ones[:1, :], WB[("bv", l, r_a)][:1, :], start=False, stop=True)
                    vsel = big.tile([128, 128], F32, tag="e_vs")
                    if two:
                        krP2 = pp_kr.tile([128, 128], F32, tag="k")
                        nc.tensor.matmul(krP2[:], WM[("mk", l, r_b)][:], xsT[:], start=True, stop=False)
                        nc.tensor.matmul(krP2[:], WB[("bk", l, r_b)][:1, :], ones[:1, :], start=False, stop=True)
                        qk2 = big.tile([128, 128], F32, tag="e_qk2")
                        nc.vector.tensor_mul(out=qk2[:], in0=qe_s[:], in1=krP2[:])
                        alP2 = pp_al.tile([128, 4], F32, tag="a")
                        nc.tensor.matmul(alP2[:], qk2[:], hm[:], start=True, stop=True)
                        asel2 = pool.tile([128, 4], F32, tag="e_as2")
                        nc.vector.tensor_scalar(out=asel2[:], in0=alP2[:], scalar1=aux[:, 2:3],
                                                scalar2=None, op0=AOP.mult)
                        nc.vector.tensor_add(out=asel[:], in0=asel[:], in1=asel2[:])
                        vrPb = pp_vr.tile([128, 128], F32, tag="v")
                        nc.tensor.matmul(vrPb[:], xsT[:], WM[("mv", l, r_b)][:], start=True, stop=False)
                        nc.tensor.matmul(vrPb[:], ones[:1, :], WB[("bv", l, r_b)][:1, :], start=False, stop=True)
                        nc.vector.tensor_scalar(out=vsel[:], in0=vrPa[:], scalar1=aux[:, 1:2],
                                                scalar2=None, op0=AOP.mult)
                        vsb = big.tile([128, 128], F32, tag="e_vsb")
                        nc.vector.tensor_scalar(out=vsb[:], in0=vrPb[:], scalar1=aux[:, 2:3],
                                                scalar2=None, op0=AOP.mult)
                        nc.vector.tensor_add(out=vsel[:], in0=vsel[:], in1=vsb[:])
                    else:
                        nc.vector.tensor_copy(out=vsel[:], in_=vrPa[:])
                    ea = pool.tile([128, 4], F32, tag="e_ea")
                    nc.scalar.activation(ea[:], asel[:], ACT.Exp)
                    nc.vector.tensor_scalar(out=ea[:], in0=ea[:], scalar1=aux[:, 3:4],
                                            scalar2=None, op0=AOP.mult)
                    wv = big.tile([128, 132], F32, tag="e_wv")
                    for h in range(H):
                        nc.vector.tensor_scalar(out=wv[:, 32 * h:32 * (h + 1)],
                                                in0=vsel[:, 32 * h:32 * (h + 1)],
                                                scalar1=ea[:, h:h + 1], scalar2=None, op0=AOP.mult)
                    nc.vector.tensor_copy(out=wv[:, 128:132], in_=ea[:])
                    aggP = pp_mm.tile([128, 132], F32, tag="m")
                    nc.tensor.matmul(aggP[:], selT[:], wv[:], start=True, stop=True)
                    agg = big.tile([128, 132], F32, tag="e_agg")
                    nc.vector.tensor_copy(out=agg[:], in_=aggP[:])
                    nc.gpsimd.indirect_dma_start(
                        out=t_u[:, :], out_offset=bass.IndirectOffsetOnAxis(ap=didx[:, :1], axis=0),
                        in_=agg[:], in_offset=None, compute_op=AOP.add,
                        bounds_check=NLP - 1, oob_is_err=False)
                tc.For_i_unrolled(0, OVT, 1, ov_body, max_unroll=2)

            # ---------- output phase ----------
            for tt in range(3):
                base_t = (B0, B1, B2)[tt]
                ntl = TYPE_P[tt] // 128
                sg_c = float(sgv[l, tt])
                dst_ap = t_x1l if l == 0 else t_out

                def out_body(i, l=l, tt=tt, base_t=base_t, sg_c=sg_c, dst_ap=dst_ap, loc_tab=loc_tab):
                    u = big.tile([128, 132], F32, tag="o_u")
                    nc.sync.dma_start(u[:], t_u[bass.ds(base_t + i * 128, 128), :])
                    dmx = pool.tile([128, 4], F32, tag="o_d")
                    nc.vector.tensor_scalar(out=dmx[:], in0=u[:, 128:132], scalar1=1e-16,
                                            scalar2=None, op0=AOP.max)
                    rec = pool.tile([128, 4], F32, tag="o_r")
                    nc.vector.reciprocal(rec[:], dmx[:])
                    aggt = big.tile([128, 128], F32, tag="o_agg")
                    for h in range(H):
                        nc.vector.tensor_scalar(out=aggt[:, 32 * h:32 * (h + 1)],
                                                in0=u[:, 32 * h:32 * (h + 1)],
                                                scalar1=rec[:, h:h + 1], scalar2=None, op0=AOP.mult)
                    agP = pp_tp.tile([128, 128], F32, tag="t")
                    nc.tensor.transpose(agP[:], aggt[:], ident[:])
                    agT = big.tile([128, 128], F32, tag="o_agT")
                    nc.scalar.activation(agT[:], agP[:], ACT.Gelu)
                    mmp = pp_mm.tile([128, 128], F32, tag="m")
                    nc.tensor.matmul(mmp[:], WM[("wout", l, tt)][:], agT[:], start=True, stop=False)
                    nc.tensor.matmul(mmp[:], WB[("bout", l, tt)][:1, :], ones[:1, :], start=False, stop=True)
                    oT = big.tile([128, 128], F32, tag="o_oT")
                    nc.vector.tensor_copy(out=oT[:], in_=mmp[:])
                    orP = pp_tp.tile([128, 128], F32, tag="t")
                    nc.tensor.transpose(orP[:], oT[:], ident[:])
                    xold = big.tile([128, 128], F32, tag="o_xo")
                    nc.sync.dma_start(xold[:], loc_tab[bass.ds(base_t + i * 128, 128), :])
                    blend = big.tile([128, 128], F32, tag="o_bl")
                    nc.vector.tensor_scalar(out=blend[:], in0=orP[:], scalar1=sg_c,
                                            scalar2=None, op0=AOP.mult)
                    xsc = big.tile([128, 128], F32, tag="o_xs")
                    nc.vector.tensor_scalar(out=xsc[:], in0=xold[:], scalar1=1.0 - sg_c,
                                            scalar2=None, op0=AOP.mult)
                    nc.vector.tensor_add(out=blend[:], in0=blend[:], in1=xsc[:])
                    nc.sync.dma_start(dst_ap[bass.ds(base_t + i * 128, 128), :], blend[:])
                tc.For_i_unrolled(0, ntl, 1, out_body, max_unroll=4)

            # ---------- AllGather after layer 0 ----------
            if l == 0:
                nc.gpsimd.collective_compute(
                    "AllGather", mybir.AluOpType.bypass,
                    replica_groups=[list(range(W))],
                    ins=[t_x1l[:, :]], outs=[t_x1f[:, :]])
        st.close()
    nc.compile()
    return nc


def kernel(**inputs):
    from concourse.bass_utils import run_bass_kernel_spmd
    prep = _host_prep(inputs)
    nc = _build(prep)
    in_maps = []
    for c in range(W):
        core = prep["cores"][c]
        m = {"x_needed": core["x_needed"], "wmats": prep["wmats"], "wbias": prep["wbias"]}
        for l in range(L):
            for gi in range(3):
                m[f"src_{l}_{gi}"] = core[f"src_{l}_{gi}"]
                m[f"aux_{l}_{gi}"] = core[f"aux_{l}_{gi}"]
                m[f"osrc_{l}_{gi}"] = core[f"osrc_{l}_{gi}"]
                m[f"oaux_{l}_{gi}"] = core[f"oaux_{l}_{gi}"]
                m[f"odst_{l}_{gi}"] = core[f"odst_{l}_{gi}"]
        in_maps.append(m)
    trace = bool(int(os.environ.get("HGT_TRACE", "0")))
    res = run_bass_kernel_spmd(nc, in_maps, list(range(W)), trace=trace)
    if trace and res.exec_time_ns:
        print(f"HW exec time: {res.exec_time_ns} ns")
    out = np.empty((P + A + K, C), np.float32)
    for c in range(W):
        o = res.results[c]["X2out"]
        out[c * PP:(c + 1) * PP] = o[B0:B0 + PP]
        out[P + c * AA:P + (c + 1) * AA] = o[B1:B1 + AA]
        out[P + A + c * KK:P + A + (c + 1) * KK] = o[B2:B2 + KK]
    return out
